# revision 34
# baseline (speedup 1.0000x reference)
"""Trainium2 Bass kernel for nn_CF_68169720922624 (segment_reduce CF predictor).

Computation (see reference):
    ub[u]   = masked mean of rating_mtx[u, :] over nonzero entries
    score[b]= sum_u  S[user[b], u] * (R[u, item[b]] - ub[u])
    out[b]  = sigmoid(score[b] + user_bias[user[b]] + item_bias[item[b]] + gb) * 5

Sharding: the contraction dim (users u) is split across 8 cores (1024 each).
Core k receives:
  r8  [8064, 2048] fp8e4 : phase-A stream; DoubleRow-interleaved transposed
                           ratings: r8[128t+p, 1024i+j] = R8[u_lo+j, 256t+128i+p]
  rt  [16128, 1152] fp16 : RT[i, j] = R[u_lo+j, i]; col 1024 = 1.0,
                           col 1025 = item_bias[i], col 1026 = gb (F gather)
  sc  [8064, 1152]  fp16 : SC[v, j] = S[v, u_lo+j]; col 1024 = user_bias[v]/8,
                           cols 1025/1026 = 1/8 (E gather)
  uw/iw [128, 512] int16 : user/item indices (item-sorted), dma_gather layout

Per core:
  Phase A: stream r8 tiles [128, 2, 1024]; mask = (r8 != 0) (split between
           DVE IS_NE and Scalar Sign to balance load vs ub latency);
           s += ones^T @ r8, c += ones^T @ mask as fp8 DoubleRow matmuls
           (256 rows per instr); ub = s/max(c,1); PE-transpose ub into
           ubT [128, 8] (u-on-partitions).
  Phase B: TRANSPOSED dma_gathers (u-on-partitions: [128, 9, 512]) of
           E rows (by user) and F rows (by item), 16 chunks alternating
           SWDGE rings so transfers overlap descriptor-gen; per chunk:
           F' = F - ubT (TS subtract, per-partition scalar, k<8 only),
           P = E*F' (TT over [128, 4608]), p[1, 512] = ones^T @ P[:,k,:]
           (9 PE matmuls accumulating in PSUM).
  Phase C: p_all [1, 8192]; AllReduce in 4 pipelined groups; sigmoid*5.
"""

import numpy as np
import ml_dtypes
from contextlib import ExitStack

import concourse.bass as bass
import concourse.bacc as bacc
import concourse.tile as tile
from concourse import mybir
from concourse.bass_utils import run_bass_kernel_spmd

F32 = mybir.dt.float32
F16 = mybir.dt.float16
F8 = mybir.dt.float8e4
I16 = mybir.dt.int16
NPF16 = np.float16
NPF8 = ml_dtypes.float8_e4m3

NCORES = 8
U = 8001
I = 16001
B = 8192
UPC = 1024          # users per core (padded; last core has 833 real)
NU = 1024           # user columns in rt/sc
D = 1152            # gathered row width: 1024 u-cols + 3 bias cols + pad
K = D // 128        # 9 k-blocks in the transposed gather layout
IP = 16128          # padded item rows (63 * 256)
SCR = 8064          # sc rows (63 * 128); only rows 0..8000 are gathered
NT8 = 63            # phase-A stream tiles of 256 interleaved rows
NB_CHUNK = 512      # idxs per dma_gather call
NCHUNK = B // NB_CHUNK          # 16

MASK_DVE_MOD = 8    # of every 8 phase-A tiles, this many masks on DVE
GE_BUFS = 5         # gather pool bufs (throttles pre-ub gather traffic)
NGROUP = 4          # AllReduce groups
GB_COLS = B // NGROUP

_CACHED = {}


def build_program(nq=4):
    """Build the SPMD bass program (identical on all 8 cores)."""
    nc = bacc.Bacc(num_devices=NCORES, num_swdge_queues=nq)

    r8 = nc.dram_tensor("r8", [NT8 * 128, 2048], F8, kind="ExternalInput")
    rt = nc.dram_tensor("rt", [IP, D], F16, kind="ExternalInput")
    sc = nc.dram_tensor("sc", [SCR, D], F16, kind="ExternalInput")
    uw = nc.dram_tensor("uw", [128, B // 16], I16, kind="ExternalInput")
    iw = nc.dram_tensor("iw", [128, B // 16], I16, kind="ExternalInput")
    out = nc.dram_tensor("out", [1, B], F32, kind="ExternalOutput")

    with ExitStack() as ctx:
        tc = ctx.enter_context(tile.TileContext(nc))
        singles = ctx.enter_context(tc.tile_pool(name="singles", bufs=1))
        r8_pool = ctx.enter_context(tc.tile_pool(name="r8_pool", bufs=4))
        m8_pool = ctx.enter_context(tc.tile_pool(name="m8_pool", bufs=4))
        psA = ctx.enter_context(tc.tile_pool(name="psA", bufs=1, space="PSUM"))
        psB = ctx.enter_context(tc.tile_pool(name="psB", bufs=2, space="PSUM"))
        ge_pool = ctx.enter_context(tc.tile_pool(name="ge_pool", bufs=GE_BUFS))
        gf_pool = ctx.enter_context(tc.tile_pool(name="gf_pool", bufs=GE_BUFS))
        fp_pool = ctx.enter_context(tc.tile_pool(name="fp_pool", bufs=2))
        pp_pool = ctx.enter_context(tc.tile_pool(name="pp_pool", bufs=2))
        pc_pool = ctx.enter_context(tc.tile_pool(name="pc_pool", bufs=2))
        dram = ctx.enter_context(tc.tile_pool(name="dram", bufs=1, space="DRAM"))

        # fp8 DoubleRow weights: [128, 2, 1] slice of a 16-col tile (the
        # k-tile stride must be 16B-aligned for the dual-fp8 LDWEIGHTS).
        ones8 = singles.tile([128, 2, 16], F8)
        nc.vector.memset(ones8, 1.0)
        ones16 = singles.tile([128, 1], F16)
        nc.vector.memset(ones16, 1.0)

        uw_sb = singles.tile([128, B // 16], I16)
        nc.sync.dma_start(uw_sb, uw[:, :])
        iw_sb = singles.tile([128, B // 16], I16)
        nc.sync.dma_start(iw_sb, iw[:, :])

        # ---- Phase A: masked-mean reductions over the core's 1024 user cols.
        # Each stream tile holds 256 item-rows DoubleRow-interleaved:
        # [128, 2, 1024] with element [p, i, j] = R8[row 256t+128i+p, col j].
        s_ps = psA.tile([1, NU], F32)
        c_ps = psA.tile([1, NU], F32)
        for t in range(NT8):
            r8t = r8_pool.tile([128, 2, NU], F8)
            nc.sync.dma_start(
                r8t, r8[t * 128:(t + 1) * 128, :].rearrange(
                    "p (i n) -> p i n", i=2)
            )
            m8 = m8_pool.tile([128, 2, NU], F8)
            if t % 8 < MASK_DVE_MOD:
                nc.vector.tensor_scalar(
                    m8, r8t, 0.0, None, mybir.AluOpType.not_equal
                )
            else:
                nc.scalar.activation(
                    m8, r8t, mybir.ActivationFunctionType.Sign
                )
            for h in range(2):
                nc.tensor.matmul(
                    s_ps[0:1, h * 512:(h + 1) * 512],
                    ones8[:, :, 0:1],
                    r8t[:, :, h * 512:(h + 1) * 512],
                    start=(t == 0),
                    stop=(t == NT8 - 1),
                    perf_mode=mybir.MatmulPerfMode.DoubleRow,
                )
                nc.tensor.matmul(
                    c_ps[0:1, h * 512:(h + 1) * 512],
                    ones8[:, :, 0:1],
                    m8[:, :, h * 512:(h + 1) * 512],
                    start=(t == 0),
                    stop=(t == NT8 - 1),
                    perf_mode=mybir.MatmulPerfMode.DoubleRow,
                )

        # ---- ub = s / max(c, 1). The row-form chain is single-partition
        # (recip on [1,1024] costs ~6.5us) but avoids the two DRAM
        # round-trips of a transposed chain (~12-20us of DMA+sem latency),
        # so it reaches ubT sooner. The [128, 8] transposed form for the
        # per-chunk subtract still needs one round-trip.
        cmax = singles.tile([1, NU], F32)
        nc.vector.tensor_scalar_max(cmax, c_ps[0:1, :], 1.0)
        crec = singles.tile([1, NU], F32)
        nc.vector.reciprocal(crec, cmax)
        ub32 = singles.tile([1, NU], F32)
        nc.vector.tensor_tensor(ub32, s_ps[0:1, :], crec, mybir.AluOpType.mult)
        ub_dram = dram.tile([1, NU], F32, name="ub_dram")
        nc.sync.dma_start(ub_dram, ub32)
        ubT = singles.tile([128, 8], F32)
        nc.sync.dma_start(
            ubT, ub_dram[0:1, :].rearrange("o (k p) -> (o p) k", k=8)
        )
        # broadcast along the 512 free columns once: ubbT[p, k, j] = ubT[p, k]
        zeros16 = singles.tile([128, NB_CHUNK], F16)
        nc.vector.memset(zeros16, 0.0)
        ubbT = singles.tile([128, 8, NB_CHUNK], F16)
        for k in range(8):
            nc.vector.tensor_scalar(
                ubbT[:, k, :], zeros16, ubT[:, k:k + 1], None,
                mybir.AluOpType.add,
            )

        # ---- Phase B: transposed gathers + folded-ub dot products.
        # p is held in one tile PER AllReduce GROUP: tile-granularity RAW
        # tracking would otherwise serialize every AllReduce behind the
        # last chunk's write.
        idx_w = NB_CHUNK // 16
        cpg = NCHUNK // NGROUP
        cc_in = dram.tile([1, B], F32, name="cci")
        for c in range(NCHUNK):
            et = ge_pool.tile([128, K, NB_CHUNK], F16, name="et")
            nc.gpsimd.dma_gather(
                et, sc[:, :], uw_sb[:, c * idx_w:(c + 1) * idx_w],
                NB_CHUNK, NB_CHUNK, D, transpose=True,
                queue_num=(2 * (c % 2)) % nq,
            )
            ft = gf_pool.tile([128, K, NB_CHUNK], F16, name="ft")
            nc.gpsimd.dma_gather(
                ft, rt[:, :], iw_sb[:, c * idx_w:(c + 1) * idx_w],
                NB_CHUNK, NB_CHUNK, D, transpose=True,
                queue_num=(2 * (c % 2) + 1) % nq,
            )
            # F' = F - ub (k < 8; the k=8 bias block needs no subtraction)
            fpt = fp_pool.tile([128, 8, NB_CHUNK], F16)
            nc.vector.tensor_tensor(
                fpt, ft[:, 0:8, :], ubbT, mybir.AluOpType.subtract
            )
            ppt = pp_pool.tile([128, K, NB_CHUNK], F16)
            nc.vector.tensor_tensor(
                ppt[:, 0:8, :], et[:, 0:8, :], fpt,
                mybir.AluOpType.mult,
            )
            nc.vector.tensor_tensor(
                ppt[:, 8, :], et[:, 8, :], ft[:, 8, :],
                mybir.AluOpType.mult,
            )
            p_ps = psB.tile([1, NB_CHUNK], F32)
            for k in range(K):
                nc.tensor.matmul(
                    p_ps[0:1, :], ones16[:, :], ppt[:, k, :],
                    start=(k == 0), stop=(k == K - 1),
                )
            ci = c % cpg
            if ci == 0:
                gstage = pc_pool.tile([1, GB_COLS], F32, name="gstage")
            nc.scalar.copy(
                gstage[0:1, ci * NB_CHUNK:(ci + 1) * NB_CHUNK], p_ps[0:1, :]
            )
            if ci == cpg - 1:
                g = c // cpg
                nc.sync.dma_start(
                    cc_in[0:1, g * GB_COLS:(g + 1) * GB_COLS], gstage
                )

        # ---- Phase C: pipelined chunked AllReduce + sigmoid * 5.
        cc_out = dram.tile([1, B], F32, name="cco")
        nc.gpsimd.collective_compute(
            "AllReduce",
            mybir.AluOpType.add,
            replica_groups=[list(range(NCORES))],
            ins=[cc_in.opt()],
            outs=[cc_out.opt()],
        )
        # 128-wide tail: the AR result comes back as [128, 64] (DRAM-side
        # rearrange), sigmoid+scale run 128 partitions wide (~0.35us each
        # instead of 4x2us single-partition), and the output DMA inverts
        # the rearrange on its DRAM destination.
        pred128 = singles.tile([128, B // 128], F32)
        nc.sync.dma_start(
            pred128, cc_out[0:1, :].rearrange("o (j p) -> (o p) j", p=128)
        )
        nc.scalar.activation(
            pred128, pred128, mybir.ActivationFunctionType.Sigmoid
        )
        nc.scalar.mul(pred128, pred128, 5.0)
        nc.sync.dma_start(
            out[0:1, :].rearrange("o (j p) -> (o p) j", p=128), pred128
        )

    nc.finalize()
    return nc


def _wrap_idxs(ix: np.ndarray) -> np.ndarray:
    """dma_gather wrapped layout: idx i of the list lives at [i % 16, i // 16],
    replicated across the eight 16-partition groups."""
    a = np.ascontiguousarray(ix.astype(np.int16).reshape(B // 16, 16).T)
    return np.ascontiguousarray(np.tile(a, (8, 1)))


def prepare_inputs(user, item, rating_mtx, user_similarity, user_bias,
                   item_bias, global_bias):
    user = np.asarray(user).astype(np.int64)
    item = np.asarray(item).astype(np.int64)
    R = np.asarray(rating_mtx, dtype=np.float32)
    S = np.asarray(user_similarity, dtype=np.float32)
    ubias = np.asarray(user_bias, dtype=np.float32)
    ibias = np.asarray(item_bias, dtype=np.float32)
    gb = np.float32(np.asarray(global_bias))

    perm = np.argsort(item, kind="stable")
    item_s = item[perm]
    user_s = user[perm]

    uw = _wrap_idxs(user_s)
    iw = _wrap_idxs(item_s)

    in_maps = []
    for k in range(NCORES):
        u_lo = k * UPC
        u_hi = min(u_lo + UPC, U)
        nu = u_hi - u_lo

        rt = np.zeros((IP, D), NPF16)
        rt[:I, :nu] = R[u_lo:u_hi, :].T.astype(NPF16)
        rt[:I, 1024] = NPF16(1.0)
        rt[:I, 1025] = ibias.astype(NPF16)
        rt[:I, 1026] = NPF16(gb)

        # phase-A fp8 stream, DoubleRow interleaved: DRAM row 128t+p holds
        # item-rows 256t+p and 256t+128+p of the transposed rating slice.
        r8full = np.zeros((IP, NU), NPF8)
        r8full[:I, :nu] = R[u_lo:u_hi, :].T.astype(NPF8)
        r8 = np.ascontiguousarray(
            r8full.reshape(NT8, 2, 128, NU).transpose(0, 2, 1, 3)
            .reshape(NT8 * 128, 2 * NU)
        )

        sc = np.zeros((SCR, D), NPF16)
        sc[:U, :nu] = S[:, u_lo:u_hi].astype(NPF16)
        sc[:U, 1024] = (ubias / np.float32(NCORES)).astype(NPF16)
        sc[:U, 1025] = NPF16(1.0 / NCORES)
        sc[:U, 1026] = NPF16(1.0 / NCORES)

        in_maps.append({"r8": r8, "rt": rt, "sc": sc, "uw": uw, "iw": iw})
    return in_maps, perm


def kernel(user, item, rating_mtx, user_similarity, user_bias, item_bias,
           global_bias, _trace=False):
    if "nc" not in _CACHED:
        _CACHED["nc"] = build_program()
    nc = _CACHED["nc"]

    in_maps, perm = prepare_inputs(
        user, item, rating_mtx, user_similarity, user_bias, item_bias,
        global_bias,
    )
    res = run_bass_kernel_spmd(nc, in_maps, core_ids=list(range(NCORES)))
    if _trace:
        # cold traced runs have hung; trace only after a warm run
        res = run_bass_kernel_spmd(
            nc, in_maps, core_ids=list(range(NCORES)), trace=True
        )
    _CACHED["last_results"] = res

    p_sorted = np.asarray(res.results[0]["out"]).reshape(-1)  # sorted-b order
    out = np.empty(B, np.float32)
    out[perm] = p_sorted
    return out


# revision 35
# speedup vs baseline: 1.1222x; 1.1222x over previous
"""Trainium2 Bass kernel for nn_CF_68169720922624 (segment_reduce CF predictor).

Computation (see reference):
    ub[u]   = masked mean of rating_mtx[u, :] over nonzero entries
    score[b]= sum_u  S[user[b], u] * (R[u, item[b]] - ub[u])
    out[b]  = sigmoid(score[b] + user_bias[user[b]] + item_bias[item[b]] + gb) * 5

Sharding: the contraction dim (users u) is split across 8 cores (1024 each).
Core k receives:
  r8  [8064, 2048] fp8e4 : phase-A stream; DoubleRow-interleaved transposed
                           ratings: r8[128t+p, 1024i+j] = R8[u_lo+j, 256t+128i+p]
  rt  [16128, 1152] fp16 : RT[i, j] = R[u_lo+j, i]; col 1024 = 1.0,
                           col 1025 = item_bias[i], col 1026 = gb (F gather)
  sc  [8064, 1152]  fp16 : SC[v, j] = S[v, u_lo+j]; col 1024 = user_bias[v]/8,
                           cols 1025/1026 = 1/8 (E gather)
  uw/iw [128, 512] int16 : user/item indices (item-sorted), dma_gather layout

Per core:
  Phase A: stream r8 tiles [128, 2, 1024]; mask = (r8 != 0) (split between
           DVE IS_NE and Scalar Sign to balance load vs ub latency);
           s += ones^T @ r8, c += ones^T @ mask as fp8 DoubleRow matmuls
           (256 rows per instr); ub = s/max(c,1); PE-transpose ub into
           ubT [128, 8] (u-on-partitions).
  Phase B: TRANSPOSED dma_gathers (u-on-partitions: [128, 9, 512]) of
           E rows (by user) and F rows (by item), 16 chunks alternating
           SWDGE rings so transfers overlap descriptor-gen; per chunk:
           F' = F - ubT (TS subtract, per-partition scalar, k<8 only),
           P = E*F' (TT over [128, 4608]), p[1, 512] = ones^T @ P[:,k,:]
           (9 PE matmuls accumulating in PSUM).
  Phase C: p_all [1, 8192]; AllReduce in 4 pipelined groups; sigmoid*5.
"""

import numpy as np
import ml_dtypes
from contextlib import ExitStack

import concourse.bass as bass
import concourse.bacc as bacc
import concourse.tile as tile
from concourse import mybir
from concourse.bass_utils import run_bass_kernel_spmd

F32 = mybir.dt.float32
F16 = mybir.dt.float16
F8 = mybir.dt.float8e4
I16 = mybir.dt.int16
NPF16 = np.float16
NPF8 = ml_dtypes.float8_e4m3

NCORES = 8
U = 8001
I = 16001
B = 8192
UPC = 1024          # users per core (padded; last core has 833 real)
NU = 1024           # user columns in rt/sc
D = 1152            # gathered row width: 1024 u-cols + 3 bias cols + pad
K = D // 128        # 9 k-blocks in the transposed gather layout
IP = 16128          # padded item rows (63 * 256)
SCR = 8064          # sc rows (63 * 128); only rows 0..8000 are gathered
NT8 = 63            # phase-A stream tiles of 256 interleaved rows
NB_CHUNK = 512      # idxs per dma_gather call
NCHUNK = B // NB_CHUNK          # 16

MASK_DVE_MOD = 8    # of every 8 phase-A tiles, this many masks on DVE
GE_BUFS = 5         # gather pool bufs (throttles pre-ub gather traffic)
NGROUP = 4          # AllReduce groups
GB_COLS = B // NGROUP

_CACHED = {}


def build_program(nq=4):
    """Build the SPMD bass program (identical on all 8 cores)."""
    nc = bacc.Bacc(num_devices=NCORES, num_swdge_queues=nq)

    r8 = nc.dram_tensor("r8", [NT8 * 128, 2048], F8, kind="ExternalInput")
    rt = nc.dram_tensor("rt", [IP, D], F16, kind="ExternalInput")
    sc = nc.dram_tensor("sc", [SCR, D], F16, kind="ExternalInput")
    uw = nc.dram_tensor("uw", [128, B // 16], I16, kind="ExternalInput")
    iw = nc.dram_tensor("iw", [128, B // 16], I16, kind="ExternalInput")
    out = nc.dram_tensor("out", [1, B], F32, kind="ExternalOutput")

    with ExitStack() as ctx:
        tc = ctx.enter_context(tile.TileContext(nc))
        singles = ctx.enter_context(tc.tile_pool(name="singles", bufs=1))
        r8_pool = ctx.enter_context(tc.tile_pool(name="r8_pool", bufs=4))
        m8_pool = ctx.enter_context(tc.tile_pool(name="m8_pool", bufs=4))
        psA = ctx.enter_context(tc.tile_pool(name="psA", bufs=1, space="PSUM"))
        psB = ctx.enter_context(tc.tile_pool(name="psB", bufs=2, space="PSUM"))
        ge_pool = ctx.enter_context(tc.tile_pool(name="ge_pool", bufs=GE_BUFS))
        gf_pool = ctx.enter_context(tc.tile_pool(name="gf_pool", bufs=GE_BUFS))
        fp_pool = ctx.enter_context(tc.tile_pool(name="fp_pool", bufs=2))
        pp_pool = ctx.enter_context(tc.tile_pool(name="pp_pool", bufs=2))
        pc_pool = ctx.enter_context(tc.tile_pool(name="pc_pool", bufs=2))
        dram = ctx.enter_context(tc.tile_pool(name="dram", bufs=1, space="DRAM"))

        # fp8 DoubleRow weights: [128, 2, 1] slice of a 16-col tile (the
        # k-tile stride must be 16B-aligned for the dual-fp8 LDWEIGHTS).
        ones8 = singles.tile([128, 2, 16], F8)
        nc.vector.memset(ones8, 1.0)
        ones16 = singles.tile([128, 1], F16)
        nc.vector.memset(ones16, 1.0)

        uw_sb = singles.tile([128, B // 16], I16)
        nc.sync.dma_start(uw_sb, uw[:, :])
        iw_sb = singles.tile([128, B // 16], I16)
        nc.sync.dma_start(iw_sb, iw[:, :])

        # ---- Phase A: masked-mean reductions over the core's 1024 user cols.
        # Each stream tile holds 256 item-rows DoubleRow-interleaved:
        # [128, 2, 1024] with element [p, i, j] = R8[row 256t+128i+p, col j].
        s_ps = psA.tile([1, NU], F32)
        c_ps = psA.tile([1, NU], F32)
        for t in range(NT8):
            r8t = r8_pool.tile([128, 2, NU], F8)
            nc.sync.dma_start(
                r8t, r8[t * 128:(t + 1) * 128, :].rearrange(
                    "p (i n) -> p i n", i=2)
            )
            m8 = m8_pool.tile([128, 2, NU], F8)
            if t % 8 < MASK_DVE_MOD:
                nc.vector.tensor_scalar(
                    m8, r8t, 0.0, None, mybir.AluOpType.not_equal
                )
            else:
                nc.scalar.activation(
                    m8, r8t, mybir.ActivationFunctionType.Sign
                )
            for h in range(2):
                nc.tensor.matmul(
                    s_ps[0:1, h * 512:(h + 1) * 512],
                    ones8[:, :, 0:1],
                    r8t[:, :, h * 512:(h + 1) * 512],
                    start=(t == 0),
                    stop=(t == NT8 - 1),
                    perf_mode=mybir.MatmulPerfMode.DoubleRow,
                )
                nc.tensor.matmul(
                    c_ps[0:1, h * 512:(h + 1) * 512],
                    ones8[:, :, 0:1],
                    m8[:, :, h * 512:(h + 1) * 512],
                    start=(t == 0),
                    stop=(t == NT8 - 1),
                    perf_mode=mybir.MatmulPerfMode.DoubleRow,
                )

        # ---- ub = s / max(c, 1). The row-form chain is single-partition
        # (recip on [1,1024] costs ~6.5us) but avoids the two DRAM
        # round-trips of a transposed chain (~12-20us of DMA+sem latency),
        # so it reaches ubT sooner. The [128, 8] transposed form for the
        # per-chunk subtract still needs one round-trip.
        cmax = singles.tile([1, NU], F32)
        nc.vector.tensor_scalar_max(cmax, c_ps[0:1, :], 1.0)
        crec = singles.tile([1, NU], F32)
        nc.vector.reciprocal(crec, cmax)
        ub32 = singles.tile([1, NU], F32)
        nc.vector.tensor_tensor(ub32, s_ps[0:1, :], crec, mybir.AluOpType.mult)
        ub_dram = dram.tile([1, NU], F32, name="ub_dram")
        nc.sync.dma_start(ub_dram, ub32)
        ubT = singles.tile([128, 8], F32)
        nc.sync.dma_start(
            ubT, ub_dram[0:1, :].rearrange("o (k p) -> (o p) k", k=8)
        )
        # broadcast along the 512 free columns once: ubbT[p, k, j] = ubT[p, k]
        zeros16 = singles.tile([128, NB_CHUNK], F16)
        nc.vector.memset(zeros16, 0.0)
        ubbT = singles.tile([128, 8, NB_CHUNK], F16)
        for k in range(8):
            nc.vector.tensor_scalar(
                ubbT[:, k, :], zeros16, ubT[:, k:k + 1], None,
                mybir.AluOpType.add,
            )

        # ---- Phase B: transposed gathers + folded-ub dot products.
        # p is held in one tile PER AllReduce GROUP: tile-granularity RAW
        # tracking would otherwise serialize every AllReduce behind the
        # last chunk's write.
        idx_w = NB_CHUNK // 16
        cpg = NCHUNK // NGROUP
        cc_in = dram.tile([1, B], F32, name="cci")
        for c in range(NCHUNK):
            et = ge_pool.tile([128, K, NB_CHUNK], F16, name="et")
            nc.gpsimd.dma_gather(
                et, sc[:, :], uw_sb[:, c * idx_w:(c + 1) * idx_w],
                NB_CHUNK, NB_CHUNK, D, transpose=True,
                queue_num=(2 * (c % 2)) % nq,
            )
            ft = gf_pool.tile([128, K, NB_CHUNK], F16, name="ft")
            nc.gpsimd.dma_gather(
                ft, rt[:, :], iw_sb[:, c * idx_w:(c + 1) * idx_w],
                NB_CHUNK, NB_CHUNK, D, transpose=True,
                queue_num=(2 * (c % 2) + 1) % nq,
            )
            # F' = F - ub (k < 8; the k=8 bias block needs no subtraction)
            fpt = fp_pool.tile([128, 8, NB_CHUNK], F16)
            nc.vector.tensor_tensor(
                fpt, ft[:, 0:8, :], ubbT, mybir.AluOpType.subtract
            )
            ppt = pp_pool.tile([128, K, NB_CHUNK], F16)
            nc.vector.tensor_tensor(
                ppt[:, 0:8, :], et[:, 0:8, :], fpt,
                mybir.AluOpType.mult,
            )
            nc.vector.tensor_tensor(
                ppt[:, 8, :], et[:, 8, :], ft[:, 8, :],
                mybir.AluOpType.mult,
            )
            p_ps = psB.tile([1, NB_CHUNK], F32)
            for k in range(K):
                nc.tensor.matmul(
                    p_ps[0:1, :], ones16[:, :], ppt[:, k, :],
                    start=(k == 0), stop=(k == K - 1),
                )
            ci = c % cpg
            if ci == 0:
                gstage = pc_pool.tile([1, GB_COLS], F32, name="gstage")
            nc.scalar.copy(
                gstage[0:1, ci * NB_CHUNK:(ci + 1) * NB_CHUNK], p_ps[0:1, :]
            )
            if ci == cpg - 1:
                g = c // cpg
                nc.sync.dma_start(
                    cc_in[0:1, g * GB_COLS:(g + 1) * GB_COLS], gstage
                )

        # ---- Phase C: pipelined chunked AllReduce + sigmoid * 5.
        cc_out = dram.tile([1, B], F32, name="cco")
        nc.gpsimd.collective_compute(
            "AllReduce",
            mybir.AluOpType.add,
            replica_groups=[list(range(NCORES))],
            ins=[cc_in.opt()],
            outs=[cc_out.opt()],
        )
        for g in range(NGROUP):
            lo, hi = g * GB_COLS, (g + 1) * GB_COLS
            pred = pc_pool.tile([1, GB_COLS], F32, name="pred")
            nc.sync.dma_start(pred, cc_out[0:1, lo:hi])
            nc.scalar.activation(
                pred, pred, mybir.ActivationFunctionType.Sigmoid
            )
            nc.scalar.mul(pred, pred, 5.0)
            nc.sync.dma_start(out[0:1, lo:hi], pred)

    nc.finalize()
    return nc


def _wrap_idxs(ix: np.ndarray) -> np.ndarray:
    """dma_gather wrapped layout: idx i of the list lives at [i % 16, i // 16],
    replicated across the eight 16-partition groups."""
    a = np.ascontiguousarray(ix.astype(np.int16).reshape(B // 16, 16).T)
    return np.ascontiguousarray(np.tile(a, (8, 1)))


def prepare_inputs(user, item, rating_mtx, user_similarity, user_bias,
                   item_bias, global_bias):
    user = np.asarray(user).astype(np.int64)
    item = np.asarray(item).astype(np.int64)
    R = np.asarray(rating_mtx, dtype=np.float32)
    S = np.asarray(user_similarity, dtype=np.float32)
    ubias = np.asarray(user_bias, dtype=np.float32)
    ibias = np.asarray(item_bias, dtype=np.float32)
    gb = np.float32(np.asarray(global_bias))

    perm = np.argsort(item, kind="stable")
    item_s = item[perm]
    user_s = user[perm]

    uw = _wrap_idxs(user_s)
    iw = _wrap_idxs(item_s)

    in_maps = []
    for k in range(NCORES):
        u_lo = k * UPC
        u_hi = min(u_lo + UPC, U)
        nu = u_hi - u_lo

        rt = np.zeros((IP, D), NPF16)
        rt[:I, :nu] = R[u_lo:u_hi, :].T.astype(NPF16)
        rt[:I, 1024] = NPF16(1.0)
        rt[:I, 1025] = ibias.astype(NPF16)
        rt[:I, 1026] = NPF16(gb)

        # phase-A fp8 stream, DoubleRow interleaved: DRAM row 128t+p holds
        # item-rows 256t+p and 256t+128+p of the transposed rating slice.
        r8full = np.zeros((IP, NU), NPF8)
        r8full[:I, :nu] = R[u_lo:u_hi, :].T.astype(NPF8)
        r8 = np.ascontiguousarray(
            r8full.reshape(NT8, 2, 128, NU).transpose(0, 2, 1, 3)
            .reshape(NT8 * 128, 2 * NU)
        )

        sc = np.zeros((SCR, D), NPF16)
        sc[:U, :nu] = S[:, u_lo:u_hi].astype(NPF16)
        sc[:U, 1024] = (ubias / np.float32(NCORES)).astype(NPF16)
        sc[:U, 1025] = NPF16(1.0 / NCORES)
        sc[:U, 1026] = NPF16(1.0 / NCORES)

        in_maps.append({"r8": r8, "rt": rt, "sc": sc, "uw": uw, "iw": iw})
    return in_maps, perm


def kernel(user, item, rating_mtx, user_similarity, user_bias, item_bias,
           global_bias, _trace=False):
    if "nc" not in _CACHED:
        _CACHED["nc"] = build_program()
    nc = _CACHED["nc"]

    in_maps, perm = prepare_inputs(
        user, item, rating_mtx, user_similarity, user_bias, item_bias,
        global_bias,
    )
    res = run_bass_kernel_spmd(nc, in_maps, core_ids=list(range(NCORES)))
    if _trace:
        # cold traced runs have hung; trace only after a warm run
        res = run_bass_kernel_spmd(
            nc, in_maps, core_ids=list(range(NCORES)), trace=True
        )
    _CACHED["last_results"] = res

    p_sorted = np.asarray(res.results[0]["out"]).reshape(-1)  # sorted-b order
    out = np.empty(B, np.float32)
    out[perm] = p_sorted
    return out


# revision 36
# speedup vs baseline: 1.1350x; 1.0114x over previous
"""Trainium2 Bass kernel for nn_CF_68169720922624 (segment_reduce CF predictor).

Computation (see reference):
    ub[u]   = masked mean of rating_mtx[u, :] over nonzero entries
    score[b]= sum_u  S[user[b], u] * (R[u, item[b]] - ub[u])
    out[b]  = sigmoid(score[b] + user_bias[user[b]] + item_bias[item[b]] + gb) * 5

Sharding: the contraction dim (users u) is split across 8 cores (1024 each).
Core k receives:
  r8  [8064, 2048] fp8e4 : phase-A stream; DoubleRow-interleaved transposed
                           ratings: r8[128t+p, 1024i+j] = R8[u_lo+j, 256t+128i+p]
  rt  [16128, 1152] fp16 : RT[i, j] = R[u_lo+j, i]; col 1024 = 1.0,
                           col 1025 = item_bias[i], col 1026 = gb (F gather)
  sc  [8064, 1152]  fp16 : SC[v, j] = S[v, u_lo+j]; col 1024 = user_bias[v]/8,
                           cols 1025/1026 = 1/8 (E gather)
  uw/iw [128, 512] int16 : user/item indices (item-sorted), dma_gather layout

Per core:
  Phase A: stream r8 tiles [128, 2, 1024]; mask = (r8 != 0) (split between
           DVE IS_NE and Scalar Sign to balance load vs ub latency);
           s += ones^T @ r8, c += ones^T @ mask as fp8 DoubleRow matmuls
           (256 rows per instr); ub = s/max(c,1); PE-transpose ub into
           ubT [128, 8] (u-on-partitions).
  Phase B: TRANSPOSED dma_gathers (u-on-partitions: [128, 9, 512]) of
           E rows (by user) and F rows (by item), 16 chunks alternating
           SWDGE rings so transfers overlap descriptor-gen; per chunk:
           F' = F - ubT (TS subtract, per-partition scalar, k<8 only),
           P = E*F' (TT over [128, 4608]), p[1, 512] = ones^T @ P[:,k,:]
           (9 PE matmuls accumulating in PSUM).
  Phase C: p_all [1, 8192]; AllReduce in 4 pipelined groups; sigmoid*5.
"""

import numpy as np
import ml_dtypes
from contextlib import ExitStack

import concourse.bass as bass
import concourse.bacc as bacc
import concourse.tile as tile
from concourse import mybir
from concourse.bass_utils import run_bass_kernel_spmd

F32 = mybir.dt.float32
F16 = mybir.dt.float16
F8 = mybir.dt.float8e4
I16 = mybir.dt.int16
NPF16 = np.float16
NPF8 = ml_dtypes.float8_e4m3

NCORES = 8
U = 8001
I = 16001
B = 8192
UPC = 1024          # users per core (padded; last core has 833 real)
NU = 1024           # user columns in rt/sc
D = 1152            # gathered row width: 1024 u-cols + 3 bias cols + pad
K = D // 128        # 9 k-blocks in the transposed gather layout
IP = 16128          # padded item rows (63 * 256)
SCR = 8064          # sc rows (63 * 128); only rows 0..8000 are gathered
NT8 = 63            # phase-A stream tiles of 256 interleaved rows
NB_CHUNK = 512      # idxs per dma_gather call
NCHUNK = B // NB_CHUNK          # 16

MASK_DVE_MOD = 8    # of every 8 phase-A tiles, this many masks on DVE
GE_BUFS = 5         # gather pool bufs (throttles pre-ub gather traffic)
NGROUP = 4          # AllReduce groups
GB_COLS = B // NGROUP

_CACHED = {}


def build_program(nq=4):
    """Build the SPMD bass program (identical on all 8 cores)."""
    nc = bacc.Bacc(num_devices=NCORES, num_swdge_queues=nq)

    r8 = nc.dram_tensor("r8", [NT8 * 128, 2048], F8, kind="ExternalInput")
    rt = nc.dram_tensor("rt", [IP, D], F16, kind="ExternalInput")
    sc = nc.dram_tensor("sc", [SCR, D], F16, kind="ExternalInput")
    uw = nc.dram_tensor("uw", [128, B // 16], I16, kind="ExternalInput")
    iw = nc.dram_tensor("iw", [128, B // 16], I16, kind="ExternalInput")
    out = nc.dram_tensor("out", [1, B], F32, kind="ExternalOutput")

    with ExitStack() as ctx:
        tc = ctx.enter_context(tile.TileContext(nc))
        singles = ctx.enter_context(tc.tile_pool(name="singles", bufs=1))
        r8_pool = ctx.enter_context(tc.tile_pool(name="r8_pool", bufs=4))
        m8_pool = ctx.enter_context(tc.tile_pool(name="m8_pool", bufs=4))
        psA = ctx.enter_context(tc.tile_pool(name="psA", bufs=1, space="PSUM"))
        psB = ctx.enter_context(tc.tile_pool(name="psB", bufs=2, space="PSUM"))
        ge_pool = ctx.enter_context(tc.tile_pool(name="ge_pool", bufs=GE_BUFS))
        gf_pool = ctx.enter_context(tc.tile_pool(name="gf_pool", bufs=GE_BUFS))
        fp_pool = ctx.enter_context(tc.tile_pool(name="fp_pool", bufs=2))
        pp_pool = ctx.enter_context(tc.tile_pool(name="pp_pool", bufs=2))
        pc_pool = ctx.enter_context(tc.tile_pool(name="pc_pool", bufs=2))
        dram = ctx.enter_context(tc.tile_pool(name="dram", bufs=1, space="DRAM"))

        # fp8 DoubleRow weights: [128, 2, 1] slice of a 16-col tile (the
        # k-tile stride must be 16B-aligned for the dual-fp8 LDWEIGHTS).
        ones8 = singles.tile([128, 2, 16], F8)
        nc.vector.memset(ones8, 1.0)
        ones16 = singles.tile([128, 1], F16)
        nc.vector.memset(ones16, 1.0)

        uw_sb = singles.tile([128, B // 16], I16)
        nc.sync.dma_start(uw_sb, uw[:, :])
        iw_sb = singles.tile([128, B // 16], I16)
        nc.sync.dma_start(iw_sb, iw[:, :])

        # ---- Phase A: masked-mean reductions over the core's 1024 user cols.
        # Each stream tile holds 256 item-rows DoubleRow-interleaved:
        # [128, 2, 1024] with element [p, i, j] = R8[row 256t+128i+p, col j].
        s_ps = psA.tile([1, NU], F32)
        c_ps = psA.tile([1, NU], F32)
        for t in range(NT8):
            r8t = r8_pool.tile([128, 2, NU], F8)
            nc.sync.dma_start(
                r8t, r8[t * 128:(t + 1) * 128, :].rearrange(
                    "p (i n) -> p i n", i=2)
            )
            m8 = m8_pool.tile([128, 2, NU], F8)
            if t % 8 < MASK_DVE_MOD:
                nc.vector.tensor_scalar(
                    m8, r8t, 0.0, None, mybir.AluOpType.not_equal
                )
            else:
                nc.scalar.activation(
                    m8, r8t, mybir.ActivationFunctionType.Sign
                )
            for h in range(2):
                nc.tensor.matmul(
                    s_ps[0:1, h * 512:(h + 1) * 512],
                    ones8[:, :, 0:1],
                    r8t[:, :, h * 512:(h + 1) * 512],
                    start=(t == 0),
                    stop=(t == NT8 - 1),
                    perf_mode=mybir.MatmulPerfMode.DoubleRow,
                )
                nc.tensor.matmul(
                    c_ps[0:1, h * 512:(h + 1) * 512],
                    ones8[:, :, 0:1],
                    m8[:, :, h * 512:(h + 1) * 512],
                    start=(t == 0),
                    stop=(t == NT8 - 1),
                    perf_mode=mybir.MatmulPerfMode.DoubleRow,
                )

        # ---- ub = s / max(c, 1). The row-form chain is single-partition
        # (recip on [1,1024] costs ~6.5us) but avoids the two DRAM
        # round-trips of a transposed chain (~12-20us of DMA+sem latency),
        # so it reaches ubT sooner. The [128, 8] transposed form for the
        # per-chunk subtract still needs one round-trip.
        cmax = singles.tile([1, NU], F32)
        nc.vector.tensor_scalar_max(cmax, c_ps[0:1, :], 1.0)
        crec = singles.tile([1, NU], F32)
        nc.vector.reciprocal(crec, cmax)
        ub32 = singles.tile([1, NU], F32)
        nc.vector.tensor_tensor(ub32, s_ps[0:1, :], crec, mybir.AluOpType.mult)
        ub_dram = dram.tile([1, NU], F32, name="ub_dram")
        nc.sync.dma_start(ub_dram, ub32)
        ubT = singles.tile([128, 8], F32)
        nc.sync.dma_start(
            ubT, ub_dram[0:1, :].rearrange("o (k p) -> (o p) k", k=8)
        )
        # broadcast along the 512 free columns once: ubbT[p, k, j] = ubT[p, k]
        zeros16 = singles.tile([128, NB_CHUNK], F16)
        nc.vector.memset(zeros16, 0.0)
        ubbT = singles.tile([128, 8, NB_CHUNK], F16)
        for k in range(8):
            nc.vector.tensor_scalar(
                ubbT[:, k, :], zeros16, ubT[:, k:k + 1], None,
                mybir.AluOpType.add,
            )

        # ---- Phase B: transposed gathers + folded-ub dot products.
        # p is held in one tile PER AllReduce GROUP: tile-granularity RAW
        # tracking would otherwise serialize every AllReduce behind the
        # last chunk's write.
        idx_w = NB_CHUNK // 16
        cpg = NCHUNK // NGROUP
        cc_in = dram.tile([1, B], F32, name="cci")
        for c in range(NCHUNK):
            et = ge_pool.tile([128, K, NB_CHUNK], F16, name="et")
            nc.gpsimd.dma_gather(
                et, sc[:, :], uw_sb[:, c * idx_w:(c + 1) * idx_w],
                NB_CHUNK, NB_CHUNK, D, transpose=True,
                queue_num=(2 * (c % 2)) % nq,
            )
            ft = gf_pool.tile([128, K, NB_CHUNK], F16, name="ft")
            nc.gpsimd.dma_gather(
                ft, rt[:, :], iw_sb[:, c * idx_w:(c + 1) * idx_w],
                NB_CHUNK, NB_CHUNK, D, transpose=True,
                queue_num=(2 * (c % 2) + 1) % nq,
            )
            # F' = F - ub (k < 8; the k=8 bias block needs no subtraction)
            fpt = fp_pool.tile([128, 8, NB_CHUNK], F16)
            nc.vector.tensor_tensor(
                fpt, ft[:, 0:8, :], ubbT, mybir.AluOpType.subtract
            )
            ppt = pp_pool.tile([128, K, NB_CHUNK], F16)
            nc.vector.tensor_tensor(
                ppt[:, 0:8, :], et[:, 0:8, :], fpt,
                mybir.AluOpType.mult,
            )
            nc.vector.tensor_tensor(
                ppt[:, 8, :], et[:, 8, :], ft[:, 8, :],
                mybir.AluOpType.mult,
            )
            p_ps = psB.tile([1, NB_CHUNK], F32)
            for k in range(K):
                nc.tensor.matmul(
                    p_ps[0:1, :], ones16[:, :], ppt[:, k, :],
                    start=(k == 0), stop=(k == K - 1),
                )
            ci = c % cpg
            if ci == 0:
                gstage = pc_pool.tile([1, GB_COLS], F32, name="gstage")
            nc.scalar.copy(
                gstage[0:1, ci * NB_CHUNK:(ci + 1) * NB_CHUNK], p_ps[0:1, :]
            )
            if ci == cpg - 1:
                g = c // cpg
                nc.sync.dma_start(
                    cc_in[0:1, g * GB_COLS:(g + 1) * GB_COLS], gstage
                )

        # ---- Phase C: pipelined chunked AllReduce + sigmoid * 5.
        cc_out = dram.tile([1, B], F32, name="cco")
        nc.gpsimd.collective_compute(
            "AllReduce",
            mybir.AluOpType.add,
            replica_groups=[list(range(NCORES))],
            ins=[cc_in.opt()],
            outs=[cc_out.opt()],
        )
        # 128-wide tail: the AR result comes back as [128, 64] (DRAM-side
        # rearrange), sigmoid+scale run 128 partitions wide (~0.35us each
        # instead of 4x2us single-partition), and the output DMA inverts
        # the rearrange on its DRAM destination.
        pred128 = singles.tile([128, B // 128], F32)
        nc.sync.dma_start(
            pred128, cc_out[0:1, :].rearrange("o (p j) -> (o p) j", p=128)
        )
        nc.scalar.activation(
            pred128, pred128, mybir.ActivationFunctionType.Sigmoid
        )
        nc.scalar.mul(pred128, pred128, 5.0)
        nc.sync.dma_start(
            out[0:1, :].rearrange("o (p j) -> (o p) j", p=128), pred128
        )

    nc.finalize()
    return nc


def _wrap_idxs(ix: np.ndarray) -> np.ndarray:
    """dma_gather wrapped layout: idx i of the list lives at [i % 16, i // 16],
    replicated across the eight 16-partition groups."""
    a = np.ascontiguousarray(ix.astype(np.int16).reshape(B // 16, 16).T)
    return np.ascontiguousarray(np.tile(a, (8, 1)))


def prepare_inputs(user, item, rating_mtx, user_similarity, user_bias,
                   item_bias, global_bias):
    user = np.asarray(user).astype(np.int64)
    item = np.asarray(item).astype(np.int64)
    R = np.asarray(rating_mtx, dtype=np.float32)
    S = np.asarray(user_similarity, dtype=np.float32)
    ubias = np.asarray(user_bias, dtype=np.float32)
    ibias = np.asarray(item_bias, dtype=np.float32)
    gb = np.float32(np.asarray(global_bias))

    perm = np.argsort(item, kind="stable")
    item_s = item[perm]
    user_s = user[perm]

    uw = _wrap_idxs(user_s)
    iw = _wrap_idxs(item_s)

    in_maps = []
    for k in range(NCORES):
        u_lo = k * UPC
        u_hi = min(u_lo + UPC, U)
        nu = u_hi - u_lo

        rt = np.zeros((IP, D), NPF16)
        rt[:I, :nu] = R[u_lo:u_hi, :].T.astype(NPF16)
        rt[:I, 1024] = NPF16(1.0)
        rt[:I, 1025] = ibias.astype(NPF16)
        rt[:I, 1026] = NPF16(gb)

        # phase-A fp8 stream, DoubleRow interleaved: DRAM row 128t+p holds
        # item-rows 256t+p and 256t+128+p of the transposed rating slice.
        r8full = np.zeros((IP, NU), NPF8)
        r8full[:I, :nu] = R[u_lo:u_hi, :].T.astype(NPF8)
        r8 = np.ascontiguousarray(
            r8full.reshape(NT8, 2, 128, NU).transpose(0, 2, 1, 3)
            .reshape(NT8 * 128, 2 * NU)
        )

        sc = np.zeros((SCR, D), NPF16)
        sc[:U, :nu] = S[:, u_lo:u_hi].astype(NPF16)
        sc[:U, 1024] = (ubias / np.float32(NCORES)).astype(NPF16)
        sc[:U, 1025] = NPF16(1.0 / NCORES)
        sc[:U, 1026] = NPF16(1.0 / NCORES)

        in_maps.append({"r8": r8, "rt": rt, "sc": sc, "uw": uw, "iw": iw})
    return in_maps, perm


def kernel(user, item, rating_mtx, user_similarity, user_bias, item_bias,
           global_bias, _trace=False):
    if "nc" not in _CACHED:
        _CACHED["nc"] = build_program()
    nc = _CACHED["nc"]

    in_maps, perm = prepare_inputs(
        user, item, rating_mtx, user_similarity, user_bias, item_bias,
        global_bias,
    )
    res = run_bass_kernel_spmd(nc, in_maps, core_ids=list(range(NCORES)))
    if _trace:
        # cold traced runs have hung; trace only after a warm run
        res = run_bass_kernel_spmd(
            nc, in_maps, core_ids=list(range(NCORES)), trace=True
        )
    _CACHED["last_results"] = res

    p_sorted = np.asarray(res.results[0]["out"]).reshape(-1)  # sorted-b order
    out = np.empty(B, np.float32)
    out[perm] = p_sorted
    return out


# revision 37
# speedup vs baseline: 1.1974x; 1.0550x over previous
"""Trainium2 Bass kernel for nn_CF_68169720922624 (segment_reduce CF predictor).

Computation (see reference):
    ub[u]   = masked mean of rating_mtx[u, :] over nonzero entries
    score[b]= sum_u  S[user[b], u] * (R[u, item[b]] - ub[u])
    out[b]  = sigmoid(score[b] + user_bias[user[b]] + item_bias[item[b]] + gb) * 5

Sharding: the contraction dim (users u) is split across 8 cores (1024 each).
Core k receives:
  r8  [8064, 2048] fp8e4 : phase-A stream; DoubleRow-interleaved transposed
                           ratings: r8[128t+p, 1024i+j] = R8[u_lo+j, 256t+128i+p]
  rt  [16128, 1152] fp16 : RT[i, j] = R[u_lo+j, i]; col 1024 = 1.0,
                           col 1025 = item_bias[i], col 1026 = gb (F gather)
  sc  [8064, 1152]  fp16 : SC[v, j] = S[v, u_lo+j]; col 1024 = user_bias[v]/8,
                           cols 1025/1026 = 1/8 (E gather)
  uw/iw [128, 512] int16 : user/item indices (item-sorted), dma_gather layout

Per core:
  Phase A: stream r8 tiles [128, 2, 1024]; mask = (r8 != 0) (split between
           DVE IS_NE and Scalar Sign to balance load vs ub latency);
           s += ones^T @ r8, c += ones^T @ mask as fp8 DoubleRow matmuls
           (256 rows per instr); ub = s/max(c,1); PE-transpose ub into
           ubT [128, 8] (u-on-partitions).
  Phase B: TRANSPOSED dma_gathers (u-on-partitions: [128, 9, 512]) of
           E rows (by user) and F rows (by item), 16 chunks alternating
           SWDGE rings so transfers overlap descriptor-gen; per chunk:
           F' = F - ubT (TS subtract, per-partition scalar, k<8 only),
           P = E*F' (TT over [128, 4608]), p[1, 512] = ones^T @ P[:,k,:]
           (9 PE matmuls accumulating in PSUM).
  Phase C: p_all [1, 8192]; AllReduce in 4 pipelined groups; sigmoid*5.
"""

import numpy as np
import ml_dtypes
from contextlib import ExitStack

import concourse.bass as bass
import concourse.bacc as bacc
import concourse.tile as tile
from concourse import mybir
from concourse.bass_utils import run_bass_kernel_spmd

F32 = mybir.dt.float32
F16 = mybir.dt.float16
F8 = mybir.dt.float8e4
I16 = mybir.dt.int16
NPF16 = np.float16
NPF8 = ml_dtypes.float8_e4m3

NCORES = 8
U = 8001
I = 16001
B = 8192
UPC = 1024          # users per core (padded; last core has 833 real)
NU = 1024           # user columns in rt/sc
D = 1152            # gathered row width: 1024 u-cols + 3 bias cols + pad
K = D // 128        # 9 k-blocks in the transposed gather layout
IP = 16128          # padded item rows (63 * 256)
SCR = 8064          # sc rows (63 * 128); only rows 0..8000 are gathered
NT8 = 63            # phase-A stream tiles of 256 interleaved rows
NB_CHUNK = 512      # idxs per dma_gather call
NCHUNK = B // NB_CHUNK          # 16

MASK_DVE_MOD = 5    # of every 8 phase-A tiles, this many masks on DVE
GE_BUFS = 5         # gather pool bufs (throttles pre-ub gather traffic)
NGROUP = 4          # AllReduce groups
GB_COLS = B // NGROUP

_CACHED = {}


def build_program(nq=4):
    """Build the SPMD bass program (identical on all 8 cores)."""
    nc = bacc.Bacc(num_devices=NCORES, num_swdge_queues=nq)

    r8 = nc.dram_tensor("r8", [NT8 * 128, 2048], F8, kind="ExternalInput")
    rt = nc.dram_tensor("rt", [IP, D], F16, kind="ExternalInput")
    sc = nc.dram_tensor("sc", [SCR, D], F16, kind="ExternalInput")
    uw = nc.dram_tensor("uw", [128, B // 16], I16, kind="ExternalInput")
    iw = nc.dram_tensor("iw", [128, B // 16], I16, kind="ExternalInput")
    out = nc.dram_tensor("out", [1, B], F32, kind="ExternalOutput")

    with ExitStack() as ctx:
        tc = ctx.enter_context(tile.TileContext(nc))
        singles = ctx.enter_context(tc.tile_pool(name="singles", bufs=1))
        r8_pool = ctx.enter_context(tc.tile_pool(name="r8_pool", bufs=6))
        m8_pool = ctx.enter_context(tc.tile_pool(name="m8_pool", bufs=6))
        psA = ctx.enter_context(tc.tile_pool(name="psA", bufs=1, space="PSUM"))
        psB = ctx.enter_context(tc.tile_pool(name="psB", bufs=2, space="PSUM"))
        ge_pool = ctx.enter_context(tc.tile_pool(name="ge_pool", bufs=GE_BUFS))
        gf_pool = ctx.enter_context(tc.tile_pool(name="gf_pool", bufs=GE_BUFS))
        fp_pool = ctx.enter_context(tc.tile_pool(name="fp_pool", bufs=2))
        pp_pool = ctx.enter_context(tc.tile_pool(name="pp_pool", bufs=2))
        pc_pool = ctx.enter_context(tc.tile_pool(name="pc_pool", bufs=2))
        dram = ctx.enter_context(tc.tile_pool(name="dram", bufs=1, space="DRAM"))

        # fp8 DoubleRow weights: [128, 2, 1] slice of a 16-col tile (the
        # k-tile stride must be 16B-aligned for the dual-fp8 LDWEIGHTS).
        ones8 = singles.tile([128, 2, 16], F8)
        nc.vector.memset(ones8, 1.0)
        ones16 = singles.tile([128, 1], F16)
        nc.vector.memset(ones16, 1.0)

        uw_sb = singles.tile([128, B // 16], I16)
        nc.sync.dma_start(uw_sb, uw[:, :])
        iw_sb = singles.tile([128, B // 16], I16)
        nc.sync.dma_start(iw_sb, iw[:, :])

        # ---- Phase A: masked-mean reductions over the core's 1024 user cols.
        # Each stream tile holds 256 item-rows DoubleRow-interleaved:
        # [128, 2, 1024] with element [p, i, j] = R8[row 256t+128i+p, col j].
        s_ps = psA.tile([1, NU], F32)
        c_ps = psA.tile([1, NU], F32)
        for t in range(NT8):
            r8t = r8_pool.tile([128, 2, NU], F8)
            nc.sync.dma_start(
                r8t, r8[t * 128:(t + 1) * 128, :].rearrange(
                    "p (i n) -> p i n", i=2)
            )
            m8 = m8_pool.tile([128, 2, NU], F8)
            if t % 8 < MASK_DVE_MOD:
                nc.vector.tensor_scalar(
                    m8, r8t, 0.0, None, mybir.AluOpType.not_equal
                )
            else:
                nc.scalar.activation(
                    m8, r8t, mybir.ActivationFunctionType.Sign
                )
            for h in range(2):
                nc.tensor.matmul(
                    s_ps[0:1, h * 512:(h + 1) * 512],
                    ones8[:, :, 0:1],
                    r8t[:, :, h * 512:(h + 1) * 512],
                    start=(t == 0),
                    stop=(t == NT8 - 1),
                    perf_mode=mybir.MatmulPerfMode.DoubleRow,
                )
                nc.tensor.matmul(
                    c_ps[0:1, h * 512:(h + 1) * 512],
                    ones8[:, :, 0:1],
                    m8[:, :, h * 512:(h + 1) * 512],
                    start=(t == 0),
                    stop=(t == NT8 - 1),
                    perf_mode=mybir.MatmulPerfMode.DoubleRow,
                )

        # ---- ub = s / max(c, 1). The row-form chain is single-partition
        # (recip on [1,1024] costs ~6.5us) but avoids the two DRAM
        # round-trips of a transposed chain (~12-20us of DMA+sem latency),
        # so it reaches ubT sooner. The [128, 8] transposed form for the
        # per-chunk subtract still needs one round-trip.
        cmax = singles.tile([1, NU], F32)
        nc.vector.tensor_scalar_max(cmax, c_ps[0:1, :], 1.0)
        crec = singles.tile([1, NU], F32)
        nc.vector.reciprocal(crec, cmax)
        ub32 = singles.tile([1, NU], F32)
        nc.vector.tensor_tensor(ub32, s_ps[0:1, :], crec, mybir.AluOpType.mult)
        ub_dram = dram.tile([1, NU], F32, name="ub_dram")
        nc.sync.dma_start(ub_dram, ub32)
        ubT = singles.tile([128, 8], F32)
        nc.sync.dma_start(
            ubT, ub_dram[0:1, :].rearrange("o (k p) -> (o p) k", k=8)
        )
        # broadcast along the 512 free columns once: ubbT[p, k, j] = ubT[p, k]
        zeros16 = singles.tile([128, NB_CHUNK], F16)
        nc.vector.memset(zeros16, 0.0)
        ubbT = singles.tile([128, 8, NB_CHUNK], F16)
        for k in range(8):
            nc.vector.tensor_scalar(
                ubbT[:, k, :], zeros16, ubT[:, k:k + 1], None,
                mybir.AluOpType.add,
            )

        # ---- Phase B: transposed gathers + folded-ub dot products.
        # p is held in one tile PER AllReduce GROUP: tile-granularity RAW
        # tracking would otherwise serialize every AllReduce behind the
        # last chunk's write.
        idx_w = NB_CHUNK // 16
        cpg = NCHUNK // NGROUP
        cc_in = dram.tile([1, B], F32, name="cci")
        for c in range(NCHUNK):
            et = ge_pool.tile([128, K, NB_CHUNK], F16, name="et")
            nc.gpsimd.dma_gather(
                et, sc[:, :], uw_sb[:, c * idx_w:(c + 1) * idx_w],
                NB_CHUNK, NB_CHUNK, D, transpose=True,
                queue_num=(2 * (c % 2)) % nq,
            )
            ft = gf_pool.tile([128, K, NB_CHUNK], F16, name="ft")
            nc.gpsimd.dma_gather(
                ft, rt[:, :], iw_sb[:, c * idx_w:(c + 1) * idx_w],
                NB_CHUNK, NB_CHUNK, D, transpose=True,
                queue_num=(2 * (c % 2) + 1) % nq,
            )
            # F' = F - ub (k < 8; the k=8 bias block needs no subtraction)
            fpt = fp_pool.tile([128, 8, NB_CHUNK], F16)
            nc.vector.tensor_tensor(
                fpt, ft[:, 0:8, :], ubbT, mybir.AluOpType.subtract
            )
            ppt = pp_pool.tile([128, K, NB_CHUNK], F16)
            nc.vector.tensor_tensor(
                ppt[:, 0:8, :], et[:, 0:8, :], fpt,
                mybir.AluOpType.mult,
            )
            nc.vector.tensor_tensor(
                ppt[:, 8, :], et[:, 8, :], ft[:, 8, :],
                mybir.AluOpType.mult,
            )
            p_ps = psB.tile([1, NB_CHUNK], F32)
            for k in range(K):
                nc.tensor.matmul(
                    p_ps[0:1, :], ones16[:, :], ppt[:, k, :],
                    start=(k == 0), stop=(k == K - 1),
                )
            ci = c % cpg
            if ci == 0:
                gstage = pc_pool.tile([1, GB_COLS], F32, name="gstage")
            nc.scalar.copy(
                gstage[0:1, ci * NB_CHUNK:(ci + 1) * NB_CHUNK], p_ps[0:1, :]
            )
            if ci == cpg - 1:
                g = c // cpg
                nc.sync.dma_start(
                    cc_in[0:1, g * GB_COLS:(g + 1) * GB_COLS], gstage
                )

        # ---- Phase C: pipelined chunked AllReduce + sigmoid * 5.
        cc_out = dram.tile([1, B], F32, name="cco")
        nc.gpsimd.collective_compute(
            "AllReduce",
            mybir.AluOpType.add,
            replica_groups=[list(range(NCORES))],
            ins=[cc_in.opt()],
            outs=[cc_out.opt()],
        )
        # 128-wide tail: the AR result comes back as [128, 64] (DRAM-side
        # rearrange), sigmoid+scale run 128 partitions wide (~0.35us each
        # instead of 4x2us single-partition), and the output DMA inverts
        # the rearrange on its DRAM destination.
        pred128 = singles.tile([128, B // 128], F32)
        nc.sync.dma_start(
            pred128, cc_out[0:1, :].rearrange("o (p j) -> (o p) j", p=128)
        )
        nc.scalar.activation(
            pred128, pred128, mybir.ActivationFunctionType.Sigmoid
        )
        nc.scalar.mul(pred128, pred128, 5.0)
        nc.sync.dma_start(
            out[0:1, :].rearrange("o (p j) -> (o p) j", p=128), pred128
        )

    nc.finalize()
    return nc


def _wrap_idxs(ix: np.ndarray) -> np.ndarray:
    """dma_gather wrapped layout: idx i of the list lives at [i % 16, i // 16],
    replicated across the eight 16-partition groups."""
    a = np.ascontiguousarray(ix.astype(np.int16).reshape(B // 16, 16).T)
    return np.ascontiguousarray(np.tile(a, (8, 1)))


def prepare_inputs(user, item, rating_mtx, user_similarity, user_bias,
                   item_bias, global_bias):
    user = np.asarray(user).astype(np.int64)
    item = np.asarray(item).astype(np.int64)
    R = np.asarray(rating_mtx, dtype=np.float32)
    S = np.asarray(user_similarity, dtype=np.float32)
    ubias = np.asarray(user_bias, dtype=np.float32)
    ibias = np.asarray(item_bias, dtype=np.float32)
    gb = np.float32(np.asarray(global_bias))

    perm = np.argsort(item, kind="stable")
    item_s = item[perm]
    user_s = user[perm]

    uw = _wrap_idxs(user_s)
    iw = _wrap_idxs(item_s)

    in_maps = []
    for k in range(NCORES):
        u_lo = k * UPC
        u_hi = min(u_lo + UPC, U)
        nu = u_hi - u_lo

        rt = np.zeros((IP, D), NPF16)
        rt[:I, :nu] = R[u_lo:u_hi, :].T.astype(NPF16)
        rt[:I, 1024] = NPF16(1.0)
        rt[:I, 1025] = ibias.astype(NPF16)
        rt[:I, 1026] = NPF16(gb)

        # phase-A fp8 stream, DoubleRow interleaved: DRAM row 128t+p holds
        # item-rows 256t+p and 256t+128+p of the transposed rating slice.
        r8full = np.zeros((IP, NU), NPF8)
        r8full[:I, :nu] = R[u_lo:u_hi, :].T.astype(NPF8)
        r8 = np.ascontiguousarray(
            r8full.reshape(NT8, 2, 128, NU).transpose(0, 2, 1, 3)
            .reshape(NT8 * 128, 2 * NU)
        )

        sc = np.zeros((SCR, D), NPF16)
        sc[:U, :nu] = S[:, u_lo:u_hi].astype(NPF16)
        sc[:U, 1024] = (ubias / np.float32(NCORES)).astype(NPF16)
        sc[:U, 1025] = NPF16(1.0 / NCORES)
        sc[:U, 1026] = NPF16(1.0 / NCORES)

        in_maps.append({"r8": r8, "rt": rt, "sc": sc, "uw": uw, "iw": iw})
    return in_maps, perm


def kernel(user, item, rating_mtx, user_similarity, user_bias, item_bias,
           global_bias, _trace=False):
    if "nc" not in _CACHED:
        _CACHED["nc"] = build_program()
    nc = _CACHED["nc"]

    in_maps, perm = prepare_inputs(
        user, item, rating_mtx, user_similarity, user_bias, item_bias,
        global_bias,
    )
    res = run_bass_kernel_spmd(nc, in_maps, core_ids=list(range(NCORES)))
    if _trace:
        # cold traced runs have hung; trace only after a warm run
        res = run_bass_kernel_spmd(
            nc, in_maps, core_ids=list(range(NCORES)), trace=True
        )
    _CACHED["last_results"] = res

    p_sorted = np.asarray(res.results[0]["out"]).reshape(-1)  # sorted-b order
    out = np.empty(B, np.float32)
    out[perm] = p_sorted
    return out


# revision 38
# speedup vs baseline: 1.2758x; 1.0655x over previous
"""Trainium2 Bass kernel for nn_CF_68169720922624 (segment_reduce CF predictor).

Computation (see reference):
    ub[u]   = masked mean of rating_mtx[u, :] over nonzero entries
    score[b]= sum_u  S[user[b], u] * (R[u, item[b]] - ub[u])
    out[b]  = sigmoid(score[b] + user_bias[user[b]] + item_bias[item[b]] + gb) * 5

Sharding: the contraction dim (users u) is split across 8 cores (1024 each).
Core k receives:
  r8  [8064, 2048] fp8e4 : phase-A stream; DoubleRow-interleaved transposed
                           ratings: r8[128t+p, 1024i+j] = R8[u_lo+j, 256t+128i+p]
  rt  [16128, 1152] fp16 : RT[i, j] = R[u_lo+j, i]; col 1024 = 1.0,
                           col 1025 = item_bias[i], col 1026 = gb (F gather)
  sc  [8064, 1152]  fp16 : SC[v, j] = S[v, u_lo+j]; col 1024 = user_bias[v]/8,
                           cols 1025/1026 = 1/8 (E gather)
  uw/iw [128, 512] int16 : user/item indices (item-sorted), dma_gather layout

Per core:
  Phase A: stream r8 tiles [128, 2, 1024]; mask = (r8 != 0) (split between
           DVE IS_NE and Scalar Sign to balance load vs ub latency);
           s += ones^T @ r8, c += ones^T @ mask as fp8 DoubleRow matmuls
           (256 rows per instr); ub = s/max(c,1); PE-transpose ub into
           ubT [128, 8] (u-on-partitions).
  Phase B: TRANSPOSED dma_gathers (u-on-partitions: [128, 9, 512]) of
           E rows (by user) and F rows (by item), 16 chunks alternating
           SWDGE rings so transfers overlap descriptor-gen; per chunk:
           F' = F - ubT (TS subtract, per-partition scalar, k<8 only),
           P = E*F' (TT over [128, 4608]), p[1, 512] = ones^T @ P[:,k,:]
           (9 PE matmuls accumulating in PSUM).
  Phase C: p_all [1, 8192]; AllReduce in 4 pipelined groups; sigmoid*5.
"""

import numpy as np
import ml_dtypes
from contextlib import ExitStack

import concourse.bass as bass
import concourse.bacc as bacc
import concourse.tile as tile
from concourse import mybir
from concourse.bass_utils import run_bass_kernel_spmd

F32 = mybir.dt.float32
F16 = mybir.dt.float16
F8 = mybir.dt.float8e4
I16 = mybir.dt.int16
NPF16 = np.float16
NPF8 = ml_dtypes.float8_e4m3

NCORES = 8
U = 8001
I = 16001
B = 8192
UPC = 1024          # users per core (padded; last core has 833 real)
NU = 1024           # user columns in rt/sc
D = 1152            # gathered row width: 1024 u-cols + 3 bias cols + pad
K = D // 128        # 9 k-blocks in the transposed gather layout
IP = 16128          # padded item rows (63 * 256)
SCR = 8064          # sc rows (63 * 128); only rows 0..8000 are gathered
NT8 = 63            # phase-A stream tiles of 256 interleaved rows
NB_CHUNK = 512      # idxs per dma_gather call
NCHUNK = B // NB_CHUNK          # 16

MASK_DVE_MOD = 5    # of every 8 phase-A tiles, this many masks on DVE
GE_BUFS = 5         # gather pool bufs (throttles pre-ub gather traffic)
NGROUP = 4          # AllReduce groups
GB_COLS = B // NGROUP

_CACHED = {}


def build_program(nq=4):
    """Build the SPMD bass program (identical on all 8 cores)."""
    nc = bacc.Bacc(num_devices=NCORES, num_swdge_queues=nq)

    r8 = nc.dram_tensor("r8", [NT8 * 128, 2048], F8, kind="ExternalInput")
    rt = nc.dram_tensor("rt", [IP, D], F16, kind="ExternalInput")
    sc = nc.dram_tensor("sc", [SCR, D], F16, kind="ExternalInput")
    uw = nc.dram_tensor("uw", [128, B // 16], I16, kind="ExternalInput")
    iw = nc.dram_tensor("iw", [128, B // 16], I16, kind="ExternalInput")
    out = nc.dram_tensor("out", [1, B], F32, kind="ExternalOutput")

    with ExitStack() as ctx:
        tc = ctx.enter_context(tile.TileContext(nc))
        singles = ctx.enter_context(tc.tile_pool(name="singles", bufs=1))
        r8_pool = ctx.enter_context(tc.tile_pool(name="r8_pool", bufs=6))
        m8_pool = ctx.enter_context(tc.tile_pool(name="m8_pool", bufs=6))
        psA = ctx.enter_context(tc.tile_pool(name="psA", bufs=1, space="PSUM"))
        psB = ctx.enter_context(tc.tile_pool(name="psB", bufs=2, space="PSUM"))
        ge_pool = ctx.enter_context(tc.tile_pool(name="ge_pool", bufs=GE_BUFS))
        gf_pool = ctx.enter_context(tc.tile_pool(name="gf_pool", bufs=GE_BUFS))
        fp_pool = ctx.enter_context(tc.tile_pool(name="fp_pool", bufs=2))
        pp_pool = ctx.enter_context(tc.tile_pool(name="pp_pool", bufs=2))
        pc_pool = ctx.enter_context(tc.tile_pool(name="pc_pool", bufs=2))
        dram = ctx.enter_context(tc.tile_pool(name="dram", bufs=1, space="DRAM"))

        # fp8 DoubleRow weights: [128, 2, 1] slice of a 16-col tile (the
        # k-tile stride must be 16B-aligned for the dual-fp8 LDWEIGHTS).
        ones8 = singles.tile([128, 2, 16], F8)
        nc.vector.memset(ones8, 1.0)
        ones16 = singles.tile([128, 1], F16)
        nc.vector.memset(ones16, 1.0)

        uw_sb = singles.tile([128, B // 16], I16)
        nc.sync.dma_start(uw_sb, uw[:, :])
        iw_sb = singles.tile([128, B // 16], I16)
        nc.sync.dma_start(iw_sb, iw[:, :])

        # ---- Phase A: masked-mean reductions over the core's 1024 user cols.
        # Each stream tile holds 256 item-rows DoubleRow-interleaved:
        # [128, 2, 1024] with element [p, i, j] = R8[row 256t+128i+p, col j].
        s_ps = psA.tile([1, NU], F32)
        c_ps = psA.tile([1, NU], F32)
        for t in range(NT8):
            r8t = r8_pool.tile([128, 2, NU], F8)
            nc.sync.dma_start(
                r8t, r8[t * 128:(t + 1) * 128, :].rearrange(
                    "p (i n) -> p i n", i=2)
            )
            m8 = m8_pool.tile([128, 2, NU], F8)
            if t % 8 < MASK_DVE_MOD:
                nc.vector.tensor_scalar(
                    m8, r8t, 0.0, None, mybir.AluOpType.not_equal
                )
            else:
                nc.scalar.activation(
                    m8, r8t, mybir.ActivationFunctionType.Sign
                )
            for h in range(2):
                nc.tensor.matmul(
                    s_ps[0:1, h * 512:(h + 1) * 512],
                    ones8[:, :, 0:1],
                    r8t[:, :, h * 512:(h + 1) * 512],
                    start=(t == 0),
                    stop=(t == NT8 - 1),
                    perf_mode=mybir.MatmulPerfMode.DoubleRow,
                )
                nc.tensor.matmul(
                    c_ps[0:1, h * 512:(h + 1) * 512],
                    ones8[:, :, 0:1],
                    m8[:, :, h * 512:(h + 1) * 512],
                    start=(t == 0),
                    stop=(t == NT8 - 1),
                    perf_mode=mybir.MatmulPerfMode.DoubleRow,
                )

        # ---- ub = s / max(c, 1) in the transposed [128, 8] layout. Same
        # two DMA hops as the row-form chain (SBUF rearrange cannot cross
        # the partition axis, so the reshape goes via DRAM) but the
        # max/recip/mult run 128 partitions wide (~0.3us) instead of on a
        # single partition (~11us, reciprocal alone 7.8us).
        sc_sb = singles.tile([1, 2 * NU], F32)
        nc.vector.tensor_copy(sc_sb[0:1, 0:NU], s_ps[0:1, :])
        nc.vector.tensor_copy(sc_sb[0:1, NU:2 * NU], c_ps[0:1, :])
        sc_dram = dram.tile([1, 2 * NU], F32, name="sc_dram")
        nc.sync.dma_start(sc_dram, sc_sb)
        scT = singles.tile([128, 2, 8], F32)
        nc.sync.dma_start(
            scT, sc_dram[0:1, :].rearrange("o (a k p) -> (o p) a k", a=2, k=8)
        )
        cmaxT = singles.tile([128, 8], F32)
        nc.vector.tensor_scalar_max(cmaxT, scT[:, 1, :], 1.0)
        crecT = singles.tile([128, 8], F32)
        nc.vector.reciprocal(crecT, cmaxT)
        ubT = singles.tile([128, 8], F32)
        nc.vector.tensor_tensor(ubT, scT[:, 0, :], crecT, mybir.AluOpType.mult)
        # broadcast along the 512 free columns once: ubbT[p, k, j] = ubT[p, k]
        zeros16 = singles.tile([128, NB_CHUNK], F16)
        nc.vector.memset(zeros16, 0.0)
        ubbT = singles.tile([128, 8, NB_CHUNK], F16)
        for k in range(8):
            nc.vector.tensor_scalar(
                ubbT[:, k, :], zeros16, ubT[:, k:k + 1], None,
                mybir.AluOpType.add,
            )

        # ---- Phase B: transposed gathers + folded-ub dot products.
        # p is held in one tile PER AllReduce GROUP: tile-granularity RAW
        # tracking would otherwise serialize every AllReduce behind the
        # last chunk's write.
        idx_w = NB_CHUNK // 16
        cpg = NCHUNK // NGROUP
        cc_in = dram.tile([1, B], F32, name="cci")
        for c in range(NCHUNK):
            et = ge_pool.tile([128, K, NB_CHUNK], F16, name="et")
            nc.gpsimd.dma_gather(
                et, sc[:, :], uw_sb[:, c * idx_w:(c + 1) * idx_w],
                NB_CHUNK, NB_CHUNK, D, transpose=True,
                queue_num=(2 * (c % 2)) % nq,
            )
            ft = gf_pool.tile([128, K, NB_CHUNK], F16, name="ft")
            nc.gpsimd.dma_gather(
                ft, rt[:, :], iw_sb[:, c * idx_w:(c + 1) * idx_w],
                NB_CHUNK, NB_CHUNK, D, transpose=True,
                queue_num=(2 * (c % 2) + 1) % nq,
            )
            # F' = F - ub (k < 8; the k=8 bias block needs no subtraction)
            fpt = fp_pool.tile([128, 8, NB_CHUNK], F16)
            nc.vector.tensor_tensor(
                fpt, ft[:, 0:8, :], ubbT, mybir.AluOpType.subtract
            )
            ppt = pp_pool.tile([128, K, NB_CHUNK], F16)
            nc.vector.tensor_tensor(
                ppt[:, 0:8, :], et[:, 0:8, :], fpt,
                mybir.AluOpType.mult,
            )
            nc.vector.tensor_tensor(
                ppt[:, 8, :], et[:, 8, :], ft[:, 8, :],
                mybir.AluOpType.mult,
            )
            p_ps = psB.tile([1, NB_CHUNK], F32)
            for k in range(K):
                nc.tensor.matmul(
                    p_ps[0:1, :], ones16[:, :], ppt[:, k, :],
                    start=(k == 0), stop=(k == K - 1),
                )
            ci = c % cpg
            if ci == 0:
                gstage = pc_pool.tile([1, GB_COLS], F32, name="gstage")
            nc.scalar.copy(
                gstage[0:1, ci * NB_CHUNK:(ci + 1) * NB_CHUNK], p_ps[0:1, :]
            )
            if ci == cpg - 1:
                g = c // cpg
                nc.sync.dma_start(
                    cc_in[0:1, g * GB_COLS:(g + 1) * GB_COLS], gstage
                )

        # ---- Phase C: pipelined chunked AllReduce + sigmoid * 5.
        cc_out = dram.tile([1, B], F32, name="cco")
        nc.gpsimd.collective_compute(
            "AllReduce",
            mybir.AluOpType.add,
            replica_groups=[list(range(NCORES))],
            ins=[cc_in.opt()],
            outs=[cc_out.opt()],
        )
        # 128-wide tail: the AR result comes back as [128, 64] (DRAM-side
        # rearrange), sigmoid+scale run 128 partitions wide (~0.35us each
        # instead of 4x2us single-partition), and the output DMA inverts
        # the rearrange on its DRAM destination.
        pred128 = singles.tile([128, B // 128], F32)
        nc.sync.dma_start(
            pred128, cc_out[0:1, :].rearrange("o (p j) -> (o p) j", p=128)
        )
        nc.scalar.activation(
            pred128, pred128, mybir.ActivationFunctionType.Sigmoid
        )
        nc.scalar.mul(pred128, pred128, 5.0)
        nc.sync.dma_start(
            out[0:1, :].rearrange("o (p j) -> (o p) j", p=128), pred128
        )

    nc.finalize()
    return nc


def _wrap_idxs(ix: np.ndarray) -> np.ndarray:
    """dma_gather wrapped layout: idx i of the list lives at [i % 16, i // 16],
    replicated across the eight 16-partition groups."""
    a = np.ascontiguousarray(ix.astype(np.int16).reshape(B // 16, 16).T)
    return np.ascontiguousarray(np.tile(a, (8, 1)))


def prepare_inputs(user, item, rating_mtx, user_similarity, user_bias,
                   item_bias, global_bias):
    user = np.asarray(user).astype(np.int64)
    item = np.asarray(item).astype(np.int64)
    R = np.asarray(rating_mtx, dtype=np.float32)
    S = np.asarray(user_similarity, dtype=np.float32)
    ubias = np.asarray(user_bias, dtype=np.float32)
    ibias = np.asarray(item_bias, dtype=np.float32)
    gb = np.float32(np.asarray(global_bias))

    perm = np.argsort(item, kind="stable")
    item_s = item[perm]
    user_s = user[perm]

    uw = _wrap_idxs(user_s)
    iw = _wrap_idxs(item_s)

    in_maps = []
    for k in range(NCORES):
        u_lo = k * UPC
        u_hi = min(u_lo + UPC, U)
        nu = u_hi - u_lo

        rt = np.zeros((IP, D), NPF16)
        rt[:I, :nu] = R[u_lo:u_hi, :].T.astype(NPF16)
        rt[:I, 1024] = NPF16(1.0)
        rt[:I, 1025] = ibias.astype(NPF16)
        rt[:I, 1026] = NPF16(gb)

        # phase-A fp8 stream, DoubleRow interleaved: DRAM row 128t+p holds
        # item-rows 256t+p and 256t+128+p of the transposed rating slice.
        r8full = np.zeros((IP, NU), NPF8)
        r8full[:I, :nu] = R[u_lo:u_hi, :].T.astype(NPF8)
        r8 = np.ascontiguousarray(
            r8full.reshape(NT8, 2, 128, NU).transpose(0, 2, 1, 3)
            .reshape(NT8 * 128, 2 * NU)
        )

        sc = np.zeros((SCR, D), NPF16)
        sc[:U, :nu] = S[:, u_lo:u_hi].astype(NPF16)
        sc[:U, 1024] = (ubias / np.float32(NCORES)).astype(NPF16)
        sc[:U, 1025] = NPF16(1.0 / NCORES)
        sc[:U, 1026] = NPF16(1.0 / NCORES)

        in_maps.append({"r8": r8, "rt": rt, "sc": sc, "uw": uw, "iw": iw})
    return in_maps, perm


def kernel(user, item, rating_mtx, user_similarity, user_bias, item_bias,
           global_bias, _trace=False):
    if "nc" not in _CACHED:
        _CACHED["nc"] = build_program()
    nc = _CACHED["nc"]

    in_maps, perm = prepare_inputs(
        user, item, rating_mtx, user_similarity, user_bias, item_bias,
        global_bias,
    )
    res = run_bass_kernel_spmd(nc, in_maps, core_ids=list(range(NCORES)))
    if _trace:
        # cold traced runs have hung; trace only after a warm run
        res = run_bass_kernel_spmd(
            nc, in_maps, core_ids=list(range(NCORES)), trace=True
        )
    _CACHED["last_results"] = res

    p_sorted = np.asarray(res.results[0]["out"]).reshape(-1)  # sorted-b order
    out = np.empty(B, np.float32)
    out[perm] = p_sorted
    return out
